# revision 33
# baseline (speedup 1.0000x reference)
"""Trainium2 Bass kernel for nn_ModalityConsisLoss (8 NeuronCores, data-parallel).

Reference computation:
    v_spa/v_seq = concat([f[:,a,:], f[:,2,:]], -1) @ W + b   for a in (0,1,3)  -> [3B, D]
    z = normalize_rows(concat([v_spa, v_seq]))               -> [6B, D]
    sim = z @ z.T ;  pos = diag pairs (i, i+3B)
    loss = sum(-pos/T) + sum(log(rowsum(exp(sim/T)) - diag)) / (6B)

Strategy (data-parallel over B):
  Each core owns B/8 = 256 batch rows -> 1536 of the 12288 z-rows
  (rows of both modalities for its batch slice, so pos pairs stay local).
  Per core, on device, per modality half (spa then seq):
    - load f shard, PE-transpose -> fT, projection matmuls -> vT half
      (the 3 pairs share f_2 @ W_bot, computed once as c2; heads 0/1
      share lhsT and run as one N=512 stream)
    - column norms via ones-matmul; r = 16/sqrt(ssq) via ACT Sqrt +
      fast approx reciprocal; gpsimd partition_broadcast spreads r
    - zT_half = fp8_e4m3(vT * r)  [512, 768]  (x16 scaling keeps fp8 in
      normal range; folded back via the exp() scale and the pos term)
    - AllGather the half. A tiny dummy AllGather at kernel start absorbs
      the one-time CC setup and aligns ranks; ag_in DMAs ride the gpsimd
      queue and the gathered-spread DMAs ride the Sync queue so neither
      AllGather trigger is blocked behind the other's dependencies.
  sim tiles: DoubleRow fp8 matmuls (K=256 per instruction) of
  zT_local.T @ zT_all with exp(sim/(T*256)) on ACT -> e (bf16, SBUF).
  Row sums alternate per chunk between ACT accum_out and a DVE reduce
  (balances the two engines; ACT's accum drain costs ~285ns/op).
  Phase C column sums accumulate e chunks on DVE and take one
  ones-matmul sweep after the loop (a per-chunk PE colsum would
  serialize PE behind ACT in the in-order PE queue), then a
  ReduceScatter recovers the seq rows' spa-column contributions.
  denom = rowsum - e^2 ; partial loss = sum(log denom) - (2/T)*sum(pos).
  Host sums the 8 partial scalars (the trivial all-reduce of the loss).
"""
import sys
from contextlib import ExitStack

sys.path.insert(0, "/opt/trn_rl_repo")

import numpy as np

import concourse.bass as bass
import concourse.mybir as mybir
import concourse.tile as tile
from concourse import bacc
from concourse import bass_utils
from concourse.masks import make_identity

F32 = mybir.dt.float32
BF16 = mybir.dt.bfloat16
FP8 = mybir.dt.float8e4
AF = mybir.ActivationFunctionType
ALU = mybir.AluOpType
DR = mybir.MatmulPerfMode.DoubleRow

N_CORES = 8
B = 2048
BL = B // N_CORES          # 256 local batch rows
D = 512
KB = D // 128              # 4 d blocks of 128
HROWS = 3 * BL             # 768 rows per modality half
LROWS = 2 * HROWS          # 1536 local z-rows (spa 768 | seq 768)
R = N_CORES * LROWS        # 12288 total rows
HALL = N_CORES * HROWS     # 6144 gathered columns per half
IB = LROWS // 128          # 12 row blocks of 128 per core
SIMW = 1536                # sim chunk width (3 PSUM banks, one ACT op)
CC = HALL // SIMW          # 3 sim column chunks per half
LH = (0, 1, 3)             # left heads of the pairs (x, 2)
TEMP = 0.5
ZSCALE = 16.0              # fp8 z scaling
ESCALE = (1.0 / TEMP) / (ZSCALE * ZSCALE)
POS_COEF = (-2.0 / TEMP) / (ZSCALE * ZSCALE)
E2 = float(np.exp(2.0))    # diagonal term exp(2 * ||z||^2), ||z|| == 1
INV_COUNT = 1.0 / R        # final 1/(2*half)


def _body(ctx, nc, tc, f_aps, w_ap, b_ap, out_ap):
    const_pool = ctx.enter_context(tc.tile_pool(name="const", bufs=1))
    small_pool = ctx.enter_context(tc.tile_pool(name="small", bufs=1))
    vt_pool = ctx.enter_context(tc.tile_pool(name="vt", bufs=1))
    dram_pool = ctx.enter_context(tc.tile_pool(name="dram", bufs=1,
                                               space="DRAM"))
    big_pool = ctx.enter_context(tc.tile_pool(name="big", bufs=1))

    ident = const_pool.tile([128, 128], F32)
    make_identity(nc, ident[:])
    # Dummy tiny AllGather issued first: it absorbs the one-time CC-path
    # setup (~11.5us trigger->start delay otherwise paid by the first real
    # collective) and tightens rank alignment, which measurably shortens
    # the spa AllGather (19-25us vs 33-37us without it).
    dummy_in = dram_pool.tile([8], F32, tag="dummy_in")
    dummy_out = dram_pool.tile([8 * N_CORES], F32, addr_space="Shared",
                               tag="dummy_out")
    nc.gpsimd.collective_compute(
        "AllGather", ALU.bypass,
        replica_groups=[list(range(N_CORES))],
        ins=[dummy_in.opt()], outs=[dummy_out.opt()])
    ones_col = const_pool.tile([128, 1], F32)
    nc.vector.memset(ones_col[:], 1.0)
    ones_row = const_pool.tile([1, 128], F32)
    nc.vector.memset(ones_row[:], 1.0)
    neg_e2 = const_pool.tile([128, 1], F32)
    nc.vector.memset(neg_e2[:], -E2)
    ln_zs = const_pool.tile([1, 1], F32)
    nc.vector.memset(ln_zs[:], float(np.log(ZSCALE)))
    # preload the sqrt table set during the idle startup window so the
    # norm chain (which gates the AllGather issue) doesn't pay the load
    nc.scalar.activation(ln_zs[:], ln_zs[:], AF.Sqrt)

    # b columns: [128, 4] (per d_out block)
    b_col = const_pool.tile([128, 4], F32)
    for m in range(KB):
        nc.sync.dma_start(b_col[:, m:m + 1], b_ap[m * 128:(m + 1) * 128])
    w_bf = const_pool.tile([128, 8, D], BF16)

    vT = vt_pool.tile([128, KB, LROWS], F32)       # [d_out(blk,128), rows]
    zT_loc = small_pool.tile([128, KB, LROWS], FP8, tag="zT_loc")
    r_row = small_pool.tile([1, LROWS], F32, tag="r_row")
    zT_all = [None, None]
    ag_outs = []

    with tc.tile_pool(name="fstage", bufs=2) as fst_pool, \
         tc.tile_pool(name="ftrans", bufs=1) as ft_pool, \
         tc.tile_pool(name="sq", bufs=2) as sq_pool, \
         tc.tile_pool(name="ps_t", bufs=2, space="PSUM") as ps_t, \
         tc.tile_pool(name="ps_proj", bufs=2, space="PSUM") as ps_proj, \
         tc.tile_pool(name="ps_s", bufs=2, space="PSUM") as ps_s:

        # PE warm-up: HAM holds the PE at 1.2 GHz until ~3.4us of sustained
        # activity; the PE would otherwise idle here waiting for f. Chained
        # dummy matmuls on zeroed data warm it so the transposes and
        # projection run at full clock. A scrap copy + WAW DMA to out keeps
        # the chain live (overwritten by the real result at the end).
        # Short PE warm-up sized to the f-h0 DMA latency (~3.5us): engages
        # the HAM busy window so the transposes+projection run at full
        # clock without delaying them (the PE queue is FIFO).
        warm_sb = const_pool.tile([128, 512], BF16)
        nc.vector.memset(warm_sb[:], 0.0)
        wps = ps_t.tile([128, 512], F32, name="wps", tag="pst")
        for _ in range(10):
            nc.tensor.matmul(wps[:], lhsT=warm_sb[:, 0:128],
                             rhs=warm_sb[:], start=True, stop=True)
        scrap = const_pool.tile([1, 1], F32)
        nc.vector.tensor_copy(scrap[:], wps[0:1, 0:1])
        nc.sync.dma_start(out_ap[:], scrap[:])

        # spa-h0 f first (transposes start the PE critical path), then W
        # (so w_bf is cast by the time the projection starts), then the
        # remaining f tiles.
        f_sts = {}

        def load_f(mod, h):
            f_st = fst_pool.tile([128, 4 * D], F32,
                                 name=f"f_st{mod}{h}", tag="f_st")
            nc.sync.dma_start(
                f_st[:], f_aps[mod][h * 128:(h + 1) * 128, :, :])
            f_sts[(mod, h)] = f_st

        load_f(0, 0)
        # W: [1024, 512] f32 -> bf16 [128, 8(kblk), 512(d_out)]; cast the
        # bottom half (kb 4-7) first -- the shared-c2 matmuls consume it
        w_st = fst_pool.tile([128, 8, D], F32, tag="w_st", bufs=1)
        for kb in range(8):
            nc.sync.dma_start(w_st[:, kb, :], w_ap[kb * 128:(kb + 1) * 128, :])
        load_f(0, 1)
        load_f(1, 0)
        load_f(1, 1)
        # cast W on the (idle) ACT engine: a DVE cast would sit in the DVE
        # FIFO ahead of the fT copies, stalling them on the W DMA
        nc.scalar.activation(w_bf[:, 4:8, :], w_st[:, 4:8, :], AF.Copy)
        nc.scalar.activation(w_bf[:, 0:4, :], w_st[:, 0:4, :], AF.Copy)

        for mod in range(2):                   # 0 = spa, 1 = seq
            c0 = mod * HROWS
            # ---- transpose f ----
            fT = ft_pool.tile([128, 4, KB, 2 * 128], BF16, name=f"fT{mod}",
                              tag=f"fT{mod}")
            for h in range(2):                 # halves of 256 local rows
                f_st = f_sts[(mod, h)]
                for a in range(4):
                    # 4 transposes into one 4-block PSUM tile, one DVE copy
                    pst = ps_t.tile([128, KB, 128], F32, name="pst",
                                    tag="pst")
                    for kb in range(KB):
                        nc.tensor.transpose(
                            pst[:, kb, :],
                            f_st[:, a * D + kb * 128: a * D + (kb + 1) * 128],
                            ident[:])
                    nc.vector.tensor_copy(
                        fT[:, a, :, h * 128:(h + 1) * 128], pst[:])
            # ---- projection: v_pa = f_pa @ W_top + (f_2 @ W_bot + b) ----
            # the 3 pairs share the right-half term; compute c2 once
            c2 = ft_pool.tile([128, KB, 2 * 128], F32, name=f"c2{mod}",
                              tag=f"c2{mod}")
            for m in range(KB):
                psc = ps_proj.tile([128, 2 * 128], F32, name="psc", tag="psv")
                for kb in range(4):
                    nc.tensor.matmul(
                        psc[:],
                        lhsT=w_bf[:, 4 + kb, m * 128:(m + 1) * 128],
                        rhs=fT[:, 2, kb, :],
                        start=(kb == 0), stop=(kb == 3))
                nc.vector.tensor_scalar_add(
                    c2[:, m, :], psc[:], b_col[:, m:m + 1])
            # heads 0 and 1 share lhsT -> pair them into one N=512 stream
            for m in range(KB):
                psv = ps_proj.tile([128, 2, 256], F32, name="psv2",
                                   tag="psv2")
                for kb in range(4):
                    nc.tensor.matmul(
                        psv[:],
                        lhsT=w_bf[:, kb, m * 128:(m + 1) * 128],
                        rhs=fT[:, 0:2, kb, :],
                        start=(kb == 0), stop=(kb == 3))
                for pa in range(2):
                    col0 = c0 + pa * 256
                    nc.vector.tensor_add(
                        vT[:, m, col0:col0 + 256], psv[:, pa, :],
                        c2[:, m, :])
            for m in range(KB):
                psv = ps_proj.tile([128, 2 * 128], F32, name="psv",
                                   tag="psv")
                for kb in range(4):
                    nc.tensor.matmul(
                        psv[:],
                        lhsT=w_bf[:, kb, m * 128:(m + 1) * 128],
                        rhs=fT[:, 3, kb, :],
                        start=(kb == 0), stop=(kb == 3))
                col0 = c0 + 2 * 256
                nc.vector.tensor_add(
                    vT[:, m, col0:col0 + 256], psv[:], c2[:, m, :])

            # ---- norms: ssq over d for this half's 768 columns ----
            ssq = small_pool.tile([1, HROWS], F32, name=f"ssq{mod}",
                                  tag=f"ssq{mod}")
            for co, cw in ((0, 512), (512, 256)):
                ps_ssq = ps_s.tile([1, 512], F32, name="ps_ssq", tag="ps_s")
                for m in range(KB):
                    sq = sq_pool.tile([128, 512], F32, name="sq", tag="sq")
                    nc.vector.tensor_mul(sq[:, :cw],
                                         vT[:, m, c0 + co:c0 + co + cw],
                                         vT[:, m, c0 + co:c0 + co + cw])
                    nc.tensor.matmul(ps_ssq[:, :cw], lhsT=ones_col[:],
                                     rhs=sq[:, :cw],
                                     start=(m == 0), stop=(m == KB - 1))
                nc.vector.tensor_copy(ssq[:, co:co + cw], ps_ssq[:, :cw])

            # r = ZSCALE / sqrt(ssq) = exp(-0.5*ln(ssq) + ln(ZSCALE))
            # r = ZSCALE/sqrt(ssq): Sqrt (scale folds the /ZSCALE^2) + DVE
            # reciprocal -- avoids the Ln<->Exp ACT table-set thrash
            srt = small_pool.tile([1, HROWS], F32, name=f"srt{mod}",
                                  tag=f"srt{mod}")
            nc.scalar.activation(srt[:], ssq[:], AF.Sqrt, 0.0,
                                 1.0 / (ZSCALE * ZSCALE))
            srt_last = srt
            nc.vector.reciprocal_approx_fast(
                out=r_row[:, c0:c0 + HROWS], in_=srt[:])

            # zT_loc half = fp8(vT * r); broadcast r across partitions on
            # the (idle) gpsimd engine instead of a PE ones-matmul
            r_bc = small_pool.tile([128, HROWS], F32, name=f"r_bc{mod}",
                                   tag=f"r_bc{mod}")
            nc.gpsimd.partition_broadcast(r_bc[:], r_row[:, c0:c0 + HROWS])
            for m in range(KB):
                nc.vector.tensor_mul(
                    zT_loc[:, m, c0:c0 + HROWS],
                    vT[:, m, c0:c0 + HROWS], r_bc[:])

            # ---- AllGather this half ----
            ag_in = dram_pool.tile([4 * 128, HROWS], FP8, name=f"ag_in{mod}",
                                   tag=f"ag_in{mod}")
            ag_out = dram_pool.tile([N_CORES * 4 * 128, HROWS], FP8,
                                    addr_space="Shared", name=f"ag_out{mod}",
                                    tag=f"ag_out{mod}")
            # gpsimd DMA queue: keeps this off the Sync queue, where the
            # scheduler parked it behind spread DMAs that wait on AG0
            nc.gpsimd.dma_start(
                out=ag_in.rearrange("(m p) c -> p m c", p=128),
                in_=zT_loc[:, :, c0:c0 + HROWS])
            nc.gpsimd.collective_compute(
                "AllGather", ALU.bypass,
                replica_groups=[list(range(N_CORES))],
                ins=[ag_in.opt()], outs=[ag_out.opt()])
            ag_outs.append(ag_out)

        # preload the exp table set during the post-prologue ACT-idle
        # window (it otherwise loads lazily on the first sim chunk's
        # critical path); reading srt pins this after the last Sqrt use
        # so the sqrt table isn't evicted mid-norm-chain
        exp_warm = small_pool.tile([1, 1], F32, tag="exp_warm")
        nc.scalar.activation(exp_warm[:], srt_last[:, 0:1], AF.Exp,
                             scale=-1.0)

        # spread the gathered halves into SBUF; deferred until after BOTH
        # AG triggers so the seq ag_in DMA isn't stuck in the Sync DMA
        # FIFO behind spa spread DMAs (which wait on AG0 completion).
        # spa odd ranks ride the otherwise-idle Scalar DMA queue so the
        # spread completes faster under the concurrent seq AllGather
        # (nothing later may ride it: the seq spread would park ahead of
        # the sim ACTIVATEs in the Scalar FIFO and stall phase A).
        for mod in range(2):
            zT_all[mod] = big_pool.tile([128, KB, HALL], FP8,
                                        name=f"zT_all{mod}", tag=f"zTa{mod}")
            for rr in range(N_CORES):
                eng = nc.scalar if (mod == 0 and rr % 2 == 1) else nc.sync
                eng.dma_start(
                    out=zT_all[mod][:, :, rr * HROWS:(rr + 1) * HROWS],
                    in_=ag_outs[mod][rr * 512:(rr + 1) * 512, :].rearrange(
                        "(m p) c -> p m c", p=128))

        # ---- pos_i = r_i * r_{i+768} * sum_d vT[d, i] * vT[d, i+768] ----
        pos_raw = small_pool.tile([1, HROWS], F32, tag="pos_raw")
        for co, cw in ((0, 512), (512, 256)):
            ps_pp = ps_s.tile([1, 512], F32, name="ps_pp", tag="ps_s")
            for m in range(KB):
                pp = sq_pool.tile([128, 512], F32, name="pp", tag="sq")
                nc.vector.tensor_mul(pp[:, :cw], vT[:, m, co:co + cw],
                                     vT[:, m, HROWS + co:HROWS + co + cw])
                nc.tensor.matmul(ps_pp[:, :cw], lhsT=ones_col[:],
                                 rhs=pp[:, :cw],
                                 start=(m == 0), stop=(m == KB - 1))
            nc.vector.tensor_copy(pos_raw[:, co:co + cw], ps_pp[:, :cw])
        rrp = small_pool.tile([1, HROWS], F32, tag="rrp")
        nc.vector.tensor_mul(rrp[:], r_row[:, 0:HROWS], r_row[:, HROWS:LROWS])
        pos_row = small_pool.tile([1, HROWS], F32, tag="pos_row")
        nc.vector.tensor_mul(pos_row[:], pos_raw[:], rrp[:])
        pos_sum = small_pool.tile([1, 1], F32, tag="pos_sum")
        nc.vector.tensor_reduce(pos_sum[:], pos_row[:],
                                axis=mybir.AxisListType.X, op=ALU.add)

    # ---------- sim tiles + fused exp/rowsum (DoubleRow fp8) ----------
    # The sim matrix is symmetric in its modality blocks:
    #   [ A  C ]   A = spa x spa, B = seq x seq, C = spa x seq.
    #   [ C' B ]
    # We never compute C': its row sums (the seq rows' spa-column denom
    # contributions) are recovered as COLUMN sums of C via ones-matmuls,
    # then summed across cores with a ReduceScatter, whose shard-per-rank
    # output is exactly this core's seq rows (SPMD-uniform by construction).
    # Cuts the exp work (the saturated ACT engine) and the sim matmuls by 25%.
    #
    # stats col layout: [ib][mod * CC + cc]; the mod0 columns of seq row
    # blocks (the dropped C' chunks) stay zero.
    HIB = IB // 2
    stats = small_pool.tile([128, 2 * IB * CC], F32, tag="stats")
    nc.vector.memset(stats[:], 0.0)
    colacc = small_pool.tile([1, HALL], F32, tag="colacc")
    ones_col_b = const_pool.tile([128, 1], BF16)
    nc.vector.memset(ones_col_b[:], 1.0)
    colden = small_pool.tile([128, HIB], F32, tag="colden")

    def sim_chunk(ps_sim, mod, ib, cc):
        ps = ps_sim.tile([128, SIMW], F32, name="ps_sim", tag="ps_sim")
        for jt in range(SIMW // 512):
            j0 = cc * SIMW + jt * 512
            for g in range(2):
                nc.tensor.matmul(
                    ps[:, jt * 512:(jt + 1) * 512],
                    lhsT=zT_loc[:, 2 * g:2 * g + 2, ib * 128:(ib + 1) * 128],
                    rhs=zT_all[mod][:, 2 * g:2 * g + 2, j0:j0 + 512],
                    start=(g == 0), stop=(g == 1), perf_mode=DR)
        return ps

    with tc.tile_pool(name="ps_sim", bufs=2, space="PSUM") as ps_sim, \
         tc.tile_pool(name="ps_cs", bufs=2, space="PSUM") as ps_cs, \
         tc.tile_pool(name="esb", bufs=3) as esb_pool:
        colacc_sb = esb_pool.tile([128, HALL], BF16, tag="colacc_sb",
                                  bufs=1)

        # exp chunk: ACT writes e (bf16) to SBUF. In A/B the row sum runs
        # as a reduce on DVE/GPSIMD (alternating; both idle there), saving
        # ACT the accum_out drain (~285ns ACTIVATION_READ_ACCUMULATOR per
        # op). In C the DVE is busy with column-sum accumulation, so C row
        # sums stay on ACT accum_out.
        def exp_chunk(ps, scol, idx=None):
            e_sb = esb_pool.tile([128, SIMW], BF16, name="e_sb", tag="e_sb")
            if idx is None or idx % 4 == 3:
                nc.scalar.activation(e_sb[:], ps[:], AF.Exp, scale=ESCALE,
                                     accum_out=stats[:, scol:scol + 1])
            else:
                nc.scalar.activation(e_sb[:], ps[:], AF.Exp, scale=ESCALE)
                nc.vector.tensor_reduce(stats[:, scol:scol + 1], e_sb[:],
                                        axis=mybir.AxisListType.X, op=ALU.add)
            return e_sb

        # phase A: spa rows x spa cols (row sums only)
        for ib in range(HIB):
            for cc in range(CC):
                ps = sim_chunk(ps_sim, 0, ib, cc)
                exp_chunk(ps, ib * 2 * CC + cc, idx=ib * CC + cc)
        # phase C: spa rows x seq cols (row sums + column sums)
        # column sums: accumulate e chunks across row blocks on DVE (bf16),
        # one ones-matmul sweep after the loop -- keeps the PE queue free of
        # matmuls that depend on ACT output (which serialized PE<->ACT).
        for ib in range(HIB):
            for cc in range(CC):
                ps = sim_chunk(ps_sim, 1, ib, cc)
                e_sb = exp_chunk(ps, ib * 2 * CC + CC + cc, idx=None)
                sl = slice(cc * SIMW, (cc + 1) * SIMW)
                if ib == 0:
                    nc.vector.tensor_copy(colacc_sb[:, sl], e_sb[:])
                else:
                    nc.vector.tensor_add(colacc_sb[:, sl],
                                         colacc_sb[:, sl], e_sb[:])
        # phase B starts with its first row block so ACT keeps streaming
        # while the C column sums (which wait on the final DVE colacc add)
        # drain through the in-order PE queue.
        def b_block(ib, idx_of=lambda ib, cc: ib * CC + cc):
            for cc in range(CC):
                ps = sim_chunk(ps_sim, 1, ib, cc)
                exp_chunk(ps, ib * 2 * CC + CC + cc, idx=idx_of(ib, cc))

        # first block rows run on ACT accum only: the DVE is still
        # draining phase C's colacc adds, which gate the column sums
        b_block(HIB, idx_of=lambda ib, cc: None)
        for j in range(HALL // 512):
            pc = ps_cs.tile([1, 512], F32, name="pc", tag="pc")
            nc.tensor.matmul(pc[:], lhsT=ones_col_b[:],
                             rhs=colacc_sb[:, j * 512:(j + 1) * 512],
                             start=True, stop=True)
            nc.vector.tensor_copy(colacc[:, j * 512:(j + 1) * 512], pc[:])
        # ReduceScatter the seq-row column contributions: rank r's output
        # shard is rows [r*768, (r+1)*768) = exactly our local seq rows.
        rs_in = dram_pool.tile([HALL], F32, tag="rs_in")
        rs_out = dram_pool.tile([HROWS], F32, tag="rs_out")
        nc.sync.dma_start(rs_in[:], colacc[:])
        nc.gpsimd.collective_compute(
            "ReduceScatter", ALU.add,
            replica_groups=[list(range(N_CORES))],
            ins=[rs_in.opt()], outs=[rs_out.opt()])
        for j in range(HIB):
            nc.sync.dma_start(colden[:, j:j + 1],
                              rs_out[j * 128:(j + 1) * 128])
        # phase B rest: seq rows x seq cols (row sums only)
        for ib in range(HIB + 1, IB):
            b_block(ib)

    # ---------- final reduction ----------
    with tc.tile_pool(name="ps_fin", bufs=1, space="PSUM") as ps_fin:
        denom = small_pool.tile([128, IB], F32, tag="denom")
        nc.vector.tensor_reduce(
            denom[:], stats.rearrange("p (i x) -> p i x", x=2 * CC),
            axis=mybir.AxisListType.X, op=ALU.add)
        # seq rows: add the ReduceScattered spa-column contributions
        nc.vector.tensor_add(denom[:, HIB:IB], denom[:, HIB:IB], colden[:])
        logd = small_pool.tile([128, IB], F32, tag="logd")
        nc.scalar.activation(logd[:], denom[:], AF.Ln, bias=neg_e2[:])
        logsum = small_pool.tile([128, 1], F32, tag="logsum")
        nc.vector.tensor_reduce(logsum[:], logd[:],
                                axis=mybir.AxisListType.X, op=ALU.add)
        fin = ps_fin.tile([1, 1], F32, tag="fin")
        nc.tensor.matmul(fin[:], lhsT=ones_col[:], rhs=logsum[:],
                         start=True, stop=True)
        res = small_pool.tile([1, 1], F32, tag="res")
        # res = (pos_sum * POS_COEF + sum(log denom)) / R
        nc.vector.scalar_tensor_tensor(res[:], pos_sum[:], POS_COEF,
                                       fin[:], op0=ALU.mult, op1=ALU.add)
        nc.vector.tensor_scalar_mul(res[:], res[:], INV_COUNT)
        nc.sync.dma_start(out_ap[:], res[:])


_NC_CACHE = None


def build_nc():
    global _NC_CACHE
    if _NC_CACHE is not None:
        return _NC_CACHE
    nc = bacc.Bacc("TRN2", target_bir_lowering=False, debug=False,
                   num_devices=N_CORES)
    f_spa = nc.dram_tensor("f_spa", [BL, 4, D], F32, kind="ExternalInput").ap()
    f_seq = nc.dram_tensor("f_seq", [BL, 4, D], F32, kind="ExternalInput").ap()
    w_ap = nc.dram_tensor("W", [2 * D, D], F32, kind="ExternalInput").ap()
    b_ap = nc.dram_tensor("b", [D], F32, kind="ExternalInput").ap()
    out_ap = nc.dram_tensor("out", [1, 1], F32, kind="ExternalOutput").ap()
    with tile.TileContext(nc) as tc, ExitStack() as ctx:
        _body(ctx, nc, tc, (f_spa, f_seq), w_ap, b_ap, out_ap)
    nc.compile()
    _NC_CACHE = nc
    return nc


def run(inputs, **kw):
    nc = build_nc()
    f_seq = np.ascontiguousarray(np.asarray(inputs["f_seq"], dtype=np.float32))
    f_spa = np.ascontiguousarray(np.asarray(inputs["f_spa"], dtype=np.float32))
    W = np.ascontiguousarray(np.asarray(inputs["W"], dtype=np.float32))
    b = np.ascontiguousarray(np.asarray(inputs["b"], dtype=np.float32))
    in_maps = []
    for c in range(N_CORES):
        sl = slice(c * BL, (c + 1) * BL)
        in_maps.append({"f_seq": np.ascontiguousarray(f_seq[sl]),
                        "f_spa": np.ascontiguousarray(f_spa[sl]),
                        "W": W, "b": b})
    try:
        res = bass_utils.run_bass_kernel_spmd(
            nc, in_maps, core_ids=list(range(N_CORES)), **kw)
    except Exception:
        # the axon terminal occasionally reports a transient
        # "device unrecoverable" on first attach; one retry clears it
        import time
        time.sleep(15)
        res = bass_utils.run_bass_kernel_spmd(
            nc, in_maps, core_ids=list(range(N_CORES)), **kw)
    total = np.float64(0.0)
    for c in range(N_CORES):
        total += np.float64(res.results[c]["out"][0, 0])
    return np.float32(total), res


def kernel(**inputs) -> np.ndarray:
    loss, _ = run(inputs)
    return np.asarray(loss, dtype=np.float32)


if __name__ == "__main__":
    rng = np.random.default_rng(0)
    inputs = {
        "f_seq": rng.standard_normal((B, 4, D), dtype=np.float32),
        "f_spa": rng.standard_normal((B, 4, D), dtype=np.float32),
        "W": (rng.standard_normal((2 * D, D), dtype=np.float32) * 0.02),
        "b": np.zeros((D,), dtype=np.float32),
    }
    print(kernel(**inputs))



# revision 39
# speedup vs baseline: 1.0477x; 1.0477x over previous
"""Trainium2 Bass kernel for nn_ModalityConsisLoss (8 NeuronCores, data-parallel).

Reference computation:
    v_spa/v_seq = concat([f[:,a,:], f[:,2,:]], -1) @ W + b   for a in (0,1,3)  -> [3B, D]
    z = normalize_rows(concat([v_spa, v_seq]))               -> [6B, D]
    sim = z @ z.T ;  pos = diag pairs (i, i+3B)
    loss = sum(-pos/T) + sum(log(rowsum(exp(sim/T)) - diag)) / (6B)

Strategy (data-parallel over B):
  Each core owns B/8 = 256 batch rows -> 1536 of the 12288 z-rows
  (rows of both modalities for its batch slice, so pos pairs stay local).
  Per core, on device, per modality half (spa then seq):
    - load f shard, PE-transpose -> fT, projection matmuls -> vT half
      (the 3 pairs share f_2 @ W_bot, computed once as c2; heads 0/1
      share lhsT and run as one N=512 stream)
    - column norms via ones-matmul; r = 16/sqrt(ssq) via ACT Sqrt +
      fast approx reciprocal; gpsimd partition_broadcast spreads r
    - zT_half = fp8_e4m3(vT * r)  [512, 768]  (x16 scaling keeps fp8 in
      normal range; folded back via the exp() scale and the pos term)
    - AllGather the half. A tiny dummy AllGather at kernel start absorbs
      the one-time CC setup and aligns ranks; ag_in DMAs ride the gpsimd
      queue and the gathered-spread DMAs ride the Sync queue so neither
      AllGather trigger is blocked behind the other's dependencies.
  sim tiles: DoubleRow fp8 matmuls (K=256 per instruction) of
  zT_local.T @ zT_all with exp(sim/(T*256)) on ACT -> e (bf16, SBUF).
  Row sums alternate per chunk between ACT accum_out and a DVE reduce
  (balances the two engines; ACT's accum drain costs ~285ns/op).
  Phase C column sums accumulate e chunks on DVE and take one
  ones-matmul sweep after the loop (a per-chunk PE colsum would
  serialize PE behind ACT in the in-order PE queue), then a
  ReduceScatter recovers the seq rows' spa-column contributions.
  denom = rowsum - e^2 ; partial loss = sum(log denom) - (2/T)*sum(pos).
  Host sums the 8 partial scalars (the trivial all-reduce of the loss).
"""
import sys
from contextlib import ExitStack

sys.path.insert(0, "/opt/trn_rl_repo")

import numpy as np

import concourse.bass as bass
import concourse.mybir as mybir
import concourse.tile as tile
from concourse import bacc
from concourse import bass_utils
from concourse.masks import make_identity

F32 = mybir.dt.float32
BF16 = mybir.dt.bfloat16
FP8 = mybir.dt.float8e4
AF = mybir.ActivationFunctionType
ALU = mybir.AluOpType
DR = mybir.MatmulPerfMode.DoubleRow

N_CORES = 8
B = 2048
BL = B // N_CORES          # 256 local batch rows
D = 512
KB = D // 128              # 4 d blocks of 128
HROWS = 3 * BL             # 768 rows per modality half
LROWS = 2 * HROWS          # 1536 local z-rows (spa 768 | seq 768)
R = N_CORES * LROWS        # 12288 total rows
HALL = N_CORES * HROWS     # 6144 gathered columns per half
IB = LROWS // 128          # 12 row blocks of 128 per core
SIMW = 1536                # sim chunk width (3 PSUM banks, one ACT op)
CC = HALL // SIMW          # 3 sim column chunks per half
LH = (0, 1, 3)             # left heads of the pairs (x, 2)
TEMP = 0.5
ZSCALE = 16.0              # fp8 z scaling
ESCALE = (1.0 / TEMP) / (ZSCALE * ZSCALE)
POS_COEF = (-2.0 / TEMP) / (ZSCALE * ZSCALE)
E2 = float(np.exp(2.0))    # diagonal term exp(2 * ||z||^2), ||z|| == 1
INV_COUNT = 1.0 / R        # final 1/(2*half)


def _body(ctx, nc, tc, f_aps, w_ap, b_ap, out_ap):
    const_pool = ctx.enter_context(tc.tile_pool(name="const", bufs=1))
    small_pool = ctx.enter_context(tc.tile_pool(name="small", bufs=1))
    vt_pool = ctx.enter_context(tc.tile_pool(name="vt", bufs=1))
    dram_pool = ctx.enter_context(tc.tile_pool(name="dram", bufs=1,
                                               space="DRAM"))
    big_pool = ctx.enter_context(tc.tile_pool(name="big", bufs=1))

    ident = const_pool.tile([128, 128], F32)
    make_identity(nc, ident[:])
    # Dummy tiny AllGather issued first: it absorbs the one-time CC-path
    # setup (~11.5us trigger->start delay otherwise paid by the first real
    # collective) and tightens rank alignment, which measurably shortens
    # the spa AllGather (19-25us vs 33-37us without it).
    dummy_in = dram_pool.tile([8], F32, tag="dummy_in")
    dummy_out = dram_pool.tile([8 * N_CORES], F32, addr_space="Shared",
                               tag="dummy_out")
    nc.gpsimd.collective_compute(
        "AllGather", ALU.bypass,
        replica_groups=[list(range(N_CORES))],
        ins=[dummy_in.opt()], outs=[dummy_out.opt()])
    ones_col = const_pool.tile([128, 1], F32)
    nc.vector.memset(ones_col[:], 1.0)
    ones_row = const_pool.tile([1, 128], F32)
    nc.vector.memset(ones_row[:], 1.0)
    neg_e2 = const_pool.tile([128, 1], F32)
    nc.vector.memset(neg_e2[:], -E2)
    ln_zs = const_pool.tile([1, 1], F32)
    nc.vector.memset(ln_zs[:], float(np.log(ZSCALE)))
    # preload the sqrt table set during the idle startup window so the
    # norm chain (which gates the AllGather issue) doesn't pay the load
    nc.scalar.activation(ln_zs[:], ln_zs[:], AF.Sqrt)

    # b columns: [128, 4] (per d_out block)
    b_col = const_pool.tile([128, 4], F32)
    for m in range(KB):
        nc.sync.dma_start(b_col[:, m:m + 1], b_ap[m * 128:(m + 1) * 128])
    w_bf = const_pool.tile([128, 8, D], BF16)

    vT = vt_pool.tile([128, KB, LROWS], F32)       # [d_out(blk,128), rows]
    zT_loc = small_pool.tile([128, KB, LROWS], FP8, tag="zT_loc")
    r_row = small_pool.tile([1, LROWS], F32, tag="r_row")
    zT_all = [None, None]
    ag_outs = []

    with tc.tile_pool(name="fstage", bufs=2) as fst_pool, \
         tc.tile_pool(name="ftrans", bufs=1) as ft_pool, \
         tc.tile_pool(name="sq", bufs=2) as sq_pool, \
         tc.tile_pool(name="ps_t", bufs=2, space="PSUM") as ps_t, \
         tc.tile_pool(name="ps_proj", bufs=2, space="PSUM") as ps_proj, \
         tc.tile_pool(name="ps_s", bufs=2, space="PSUM") as ps_s:

        # PE warm-up: HAM holds the PE at 1.2 GHz until ~3.4us of sustained
        # activity; the PE would otherwise idle here waiting for f. Chained
        # dummy matmuls on zeroed data warm it so the transposes and
        # projection run at full clock. A scrap copy + WAW DMA to out keeps
        # the chain live (overwritten by the real result at the end).
        # Short PE warm-up sized to the f-h0 DMA latency (~3.5us): engages
        # the HAM busy window so the transposes+projection run at full
        # clock without delaying them (the PE queue is FIFO).
        warm_sb = const_pool.tile([128, 512], BF16)
        nc.vector.memset(warm_sb[:], 0.0)
        wps = ps_t.tile([128, 512], F32, name="wps", tag="pst")
        for _ in range(10):
            nc.tensor.matmul(wps[:], lhsT=warm_sb[:, 0:128],
                             rhs=warm_sb[:], start=True, stop=True)
        scrap = const_pool.tile([1, 1], F32)
        nc.vector.tensor_copy(scrap[:], wps[0:1, 0:1])
        nc.sync.dma_start(out_ap[:], scrap[:])

        # spa-h0 f first (transposes start the PE critical path), then W
        # (so w_bf is cast by the time the projection starts), then the
        # remaining f tiles.
        f_sts = {}

        def load_f(mod, h):
            f_st = fst_pool.tile([128, 4 * D], F32,
                                 name=f"f_st{mod}{h}", tag="f_st")
            nc.sync.dma_start(
                f_st[:], f_aps[mod][h * 128:(h + 1) * 128, :, :])
            f_sts[(mod, h)] = f_st

        load_f(0, 0)
        # W: [1024, 512] f32 -> bf16 [128, 8(kblk), 512(d_out)]; cast the
        # bottom half (kb 4-7) first -- the shared-c2 matmuls consume it
        w_st = fst_pool.tile([128, 8, D], F32, tag="w_st", bufs=1)
        for kb in range(8):
            nc.sync.dma_start(w_st[:, kb, :], w_ap[kb * 128:(kb + 1) * 128, :])
        load_f(0, 1)
        load_f(1, 0)
        load_f(1, 1)
        # cast W on the (idle) ACT engine: a DVE cast would sit in the DVE
        # FIFO ahead of the fT copies, stalling them on the W DMA
        nc.scalar.activation(w_bf[:, 4:8, :], w_st[:, 4:8, :], AF.Copy)
        nc.scalar.activation(w_bf[:, 0:4, :], w_st[:, 0:4, :], AF.Copy)

        for mod in range(2):                   # 0 = spa, 1 = seq
            c0 = mod * HROWS
            # ---- transpose f ----
            fT = ft_pool.tile([128, 4, KB, 2 * 128], BF16, name=f"fT{mod}",
                              tag=f"fT{mod}")
            for h in range(2):                 # halves of 256 local rows
                f_st = f_sts[(mod, h)]
                for a in range(4):
                    # 4 transposes into one 4-block PSUM tile, one DVE copy
                    pst = ps_t.tile([128, KB, 128], F32, name="pst",
                                    tag="pst")
                    for kb in range(KB):
                        nc.tensor.transpose(
                            pst[:, kb, :],
                            f_st[:, a * D + kb * 128: a * D + (kb + 1) * 128],
                            ident[:])
                    nc.vector.tensor_copy(
                        fT[:, a, :, h * 128:(h + 1) * 128], pst[:])
            # ---- projection: v_pa = f_pa @ W_top + (f_2 @ W_bot + b) ----
            # the 3 pairs share the right-half term; compute c2 once
            c2 = ft_pool.tile([128, KB, 2 * 128], F32, name=f"c2{mod}",
                              tag=f"c2{mod}")
            for m in range(KB):
                psc = ps_proj.tile([128, 2 * 128], F32, name="psc", tag="psv")
                for kb in range(4):
                    nc.tensor.matmul(
                        psc[:],
                        lhsT=w_bf[:, 4 + kb, m * 128:(m + 1) * 128],
                        rhs=fT[:, 2, kb, :],
                        start=(kb == 0), stop=(kb == 3))
                nc.vector.tensor_scalar_add(
                    c2[:, m, :], psc[:], b_col[:, m:m + 1])
            # heads 0 and 1 share lhsT -> pair them into one N=512 stream
            for m in range(KB):
                psv = ps_proj.tile([128, 2, 256], F32, name="psv2",
                                   tag="psv2")
                for kb in range(4):
                    nc.tensor.matmul(
                        psv[:],
                        lhsT=w_bf[:, kb, m * 128:(m + 1) * 128],
                        rhs=fT[:, 0:2, kb, :],
                        start=(kb == 0), stop=(kb == 3))
                for pa in range(2):
                    col0 = c0 + pa * 256
                    nc.vector.tensor_add(
                        vT[:, m, col0:col0 + 256], psv[:, pa, :],
                        c2[:, m, :])
            for m in range(KB):
                psv = ps_proj.tile([128, 2 * 128], F32, name="psv",
                                   tag="psv")
                for kb in range(4):
                    nc.tensor.matmul(
                        psv[:],
                        lhsT=w_bf[:, kb, m * 128:(m + 1) * 128],
                        rhs=fT[:, 3, kb, :],
                        start=(kb == 0), stop=(kb == 3))
                col0 = c0 + 2 * 256
                nc.vector.tensor_add(
                    vT[:, m, col0:col0 + 256], psv[:], c2[:, m, :])

            # ---- norms: ssq over d for this half's 768 columns ----
            ssq = small_pool.tile([1, HROWS], F32, name=f"ssq{mod}",
                                  tag=f"ssq{mod}")
            for co, cw in ((0, 512), (512, 256)):
                ps_ssq = ps_s.tile([1, 512], F32, name="ps_ssq", tag="ps_s")
                for m in range(KB):
                    sq = sq_pool.tile([128, 512], F32, name="sq", tag="sq")
                    nc.vector.tensor_mul(sq[:, :cw],
                                         vT[:, m, c0 + co:c0 + co + cw],
                                         vT[:, m, c0 + co:c0 + co + cw])
                    nc.tensor.matmul(ps_ssq[:, :cw], lhsT=ones_col[:],
                                     rhs=sq[:, :cw],
                                     start=(m == 0), stop=(m == KB - 1))
                nc.vector.tensor_copy(ssq[:, co:co + cw], ps_ssq[:, :cw])

            # r = ZSCALE / sqrt(ssq) = exp(-0.5*ln(ssq) + ln(ZSCALE))
            # r = ZSCALE/sqrt(ssq): Sqrt (scale folds the /ZSCALE^2) + DVE
            # reciprocal -- avoids the Ln<->Exp ACT table-set thrash
            srt = small_pool.tile([1, HROWS], F32, name=f"srt{mod}",
                                  tag=f"srt{mod}")
            nc.scalar.activation(srt[:], ssq[:], AF.Sqrt, 0.0,
                                 1.0 / (ZSCALE * ZSCALE))
            srt_last = srt
            nc.vector.reciprocal_approx_fast(
                out=r_row[:, c0:c0 + HROWS], in_=srt[:])

            # zT_loc half = fp8(vT * r); broadcast r across partitions on
            # the (idle) gpsimd engine instead of a PE ones-matmul
            r_bc = small_pool.tile([128, HROWS], F32, name=f"r_bc{mod}",
                                   tag=f"r_bc{mod}")
            nc.gpsimd.partition_broadcast(r_bc[:], r_row[:, c0:c0 + HROWS])
            for m in range(KB):
                nc.vector.tensor_mul(
                    zT_loc[:, m, c0:c0 + HROWS],
                    vT[:, m, c0:c0 + HROWS], r_bc[:])

            # ---- AllGather this half ----
            ag_in = dram_pool.tile([4 * 128, HROWS], FP8, name=f"ag_in{mod}",
                                   tag=f"ag_in{mod}")
            ag_out = dram_pool.tile([N_CORES * 4 * 128, HROWS], FP8,
                                    addr_space="Shared", name=f"ag_out{mod}",
                                    tag=f"ag_out{mod}")
            # gpsimd DMA queue: keeps this off the Sync queue, where the
            # scheduler parked it behind spread DMAs that wait on AG0
            nc.gpsimd.dma_start(
                out=ag_in.rearrange("(m p) c -> p m c", p=128),
                in_=zT_loc[:, :, c0:c0 + HROWS])
            nc.gpsimd.collective_compute(
                "AllGather", ALU.bypass,
                replica_groups=[list(range(N_CORES))],
                ins=[ag_in.opt()], outs=[ag_out.opt()])
            ag_outs.append(ag_out)

        # preload the exp table set during the post-prologue ACT-idle
        # window (it otherwise loads lazily on the first sim chunk's
        # critical path); reading srt pins this after the last Sqrt use
        exp_warm = small_pool.tile([1, 1], F32, tag="exp_warm")
        nc.scalar.activation(exp_warm[:], srt_last[:, 0:1], AF.Exp,
                             scale=-1.0)

        # spread the gathered halves into SBUF; deferred until after BOTH
        # AG triggers so the seq ag_in DMA isn't stuck in the Sync DMA
        # FIFO behind spa spread DMAs (which wait on AG0 completion).
        # spa odd ranks ride the otherwise-idle Scalar DMA queue (the seq
        # spread must not: it would park ahead of the sim ACTIVATEs in
        # the Scalar FIFO and stall phase A).
        for mod in range(2):
            zT_all[mod] = big_pool.tile([128, KB, HALL], FP8,
                                        name=f"zT_all{mod}", tag=f"zTa{mod}")
            for rr in range(N_CORES):
                eng = nc.scalar if (mod == 0 and rr % 2 == 1) else nc.sync
                eng.dma_start(
                    out=zT_all[mod][:, :, rr * HROWS:(rr + 1) * HROWS],
                    in_=ag_outs[mod][rr * 512:(rr + 1) * 512, :].rearrange(
                        "(m p) c -> p m c", p=128))

        # ---- pos_i = r_i * r_{i+768} * sum_d vT[d, i] * vT[d, i+768] ----
        pos_raw = small_pool.tile([1, HROWS], F32, tag="pos_raw")
        for co, cw in ((0, 512), (512, 256)):
            ps_pp = ps_s.tile([1, 512], F32, name="ps_pp", tag="ps_s")
            for m in range(KB):
                pp = sq_pool.tile([128, 512], F32, name="pp", tag="sq")
                nc.vector.tensor_mul(pp[:, :cw], vT[:, m, co:co + cw],
                                     vT[:, m, HROWS + co:HROWS + co + cw])
                nc.tensor.matmul(ps_pp[:, :cw], lhsT=ones_col[:],
                                 rhs=pp[:, :cw],
                                 start=(m == 0), stop=(m == KB - 1))
            nc.vector.tensor_copy(pos_raw[:, co:co + cw], ps_pp[:, :cw])
        rrp = small_pool.tile([1, HROWS], F32, tag="rrp")
        nc.vector.tensor_mul(rrp[:], r_row[:, 0:HROWS], r_row[:, HROWS:LROWS])
        pos_row = small_pool.tile([1, HROWS], F32, tag="pos_row")
        nc.vector.tensor_mul(pos_row[:], pos_raw[:], rrp[:])
        pos_sum = small_pool.tile([1, 1], F32, tag="pos_sum")
        nc.vector.tensor_reduce(pos_sum[:], pos_row[:],
                                axis=mybir.AxisListType.X, op=ALU.add)

    # ---------- sim tiles + fused exp/rowsum (DoubleRow fp8) ----------
    # The sim matrix is symmetric in its modality blocks:
    #   [ A  C ]   A = spa x spa, B = seq x seq, C = spa x seq.
    #   [ C' B ]
    # We never compute C': its row sums (the seq rows' spa-column denom
    # contributions) are recovered as COLUMN sums of C via ones-matmuls,
    # then summed across cores with a ReduceScatter, whose shard-per-rank
    # output is exactly this core's seq rows (SPMD-uniform by construction).
    # Cuts the exp work (the saturated ACT engine) and the sim matmuls by 25%.
    #
    # stats col layout: [ib][mod * CC + cc]; the mod0 columns of seq row
    # blocks (the dropped C' chunks) stay zero.
    HIB = IB // 2
    stats = small_pool.tile([128, 2 * IB * CC], F32, tag="stats")
    nc.vector.memset(stats[:], 0.0)
    colacc = small_pool.tile([1, HALL], F32, tag="colacc")
    ones_col_b = const_pool.tile([128, 1], BF16)
    nc.vector.memset(ones_col_b[:], 1.0)
    colden = small_pool.tile([128, HIB], F32, tag="colden")

    def sim_chunk(ps_sim, mod, ib, cc):
        ps = ps_sim.tile([128, SIMW], F32, name="ps_sim", tag="ps_sim")
        for jt in range(SIMW // 512):
            j0 = cc * SIMW + jt * 512
            for g in range(2):
                nc.tensor.matmul(
                    ps[:, jt * 512:(jt + 1) * 512],
                    lhsT=zT_loc[:, 2 * g:2 * g + 2, ib * 128:(ib + 1) * 128],
                    rhs=zT_all[mod][:, 2 * g:2 * g + 2, j0:j0 + 512],
                    start=(g == 0), stop=(g == 1), perf_mode=DR)
        return ps

    with tc.tile_pool(name="ps_sim", bufs=2, space="PSUM") as ps_sim, \
         tc.tile_pool(name="ps_cs", bufs=2, space="PSUM") as ps_cs, \
         tc.tile_pool(name="esb", bufs=3) as esb_pool:
        colacc_sb = esb_pool.tile([128, HALL], BF16, tag="colacc_sb",
                                  bufs=1)

        # exp chunk: ACT writes e (bf16) to SBUF. In A/B the row sum runs
        # as a reduce on DVE/GPSIMD (alternating; both idle there), saving
        # ACT the accum_out drain (~285ns ACTIVATION_READ_ACCUMULATOR per
        # op). In C the DVE is busy with column-sum accumulation, so C row
        # sums stay on ACT accum_out.
        def exp_chunk(ps, scol, idx=None):
            e_sb = esb_pool.tile([128, SIMW], BF16, name="e_sb", tag="e_sb")
            if idx is None or idx % 4 == 3:
                nc.scalar.activation(e_sb[:], ps[:], AF.Exp, scale=ESCALE,
                                     accum_out=stats[:, scol:scol + 1])
            else:
                nc.scalar.activation(e_sb[:], ps[:], AF.Exp, scale=ESCALE)
                nc.vector.tensor_reduce(stats[:, scol:scol + 1], e_sb[:],
                                        axis=mybir.AxisListType.X, op=ALU.add)
            return e_sb

        # phase A: spa rows x spa cols (row sums only)
        for ib in range(HIB):
            for cc in range(CC):
                ps = sim_chunk(ps_sim, 0, ib, cc)
                exp_chunk(ps, ib * 2 * CC + cc, idx=ib * CC + cc)
        # phase C: spa rows x seq cols (row sums + column sums)
        # column sums: accumulate e chunks across row blocks on DVE (bf16),
        # one ones-matmul sweep after the loop -- keeps the PE queue free of
        # matmuls that depend on ACT output (which serialized PE<->ACT).
        for ib in range(HIB):
            for cc in range(CC):
                ps = sim_chunk(ps_sim, 1, ib, cc)
                e_sb = exp_chunk(ps, ib * 2 * CC + CC + cc, idx=None)
                sl = slice(cc * SIMW, (cc + 1) * SIMW)
                if ib == 0:
                    nc.vector.tensor_copy(colacc_sb[:, sl], e_sb[:])
                else:
                    nc.vector.tensor_add(colacc_sb[:, sl],
                                         colacc_sb[:, sl], e_sb[:])
        # phase B starts with its first row block so ACT keeps streaming
        # while the C column sums (which wait on the final DVE colacc add)
        # drain through the in-order PE queue; that block runs ACT-accum
        # only since the DVE is still draining phase C.
        def b_block(ib, idx_of=lambda ib, cc: ib * CC + cc):
            for cc in range(CC):
                ps = sim_chunk(ps_sim, 1, ib, cc)
                exp_chunk(ps, ib * 2 * CC + CC + cc, idx=idx_of(ib, cc))

        b_block(HIB, idx_of=lambda ib, cc: None)
        for j in range(HALL // 512):
            pc = ps_cs.tile([1, 512], F32, name="pc", tag="pc")
            nc.tensor.matmul(pc[:], lhsT=ones_col_b[:],
                             rhs=colacc_sb[:, j * 512:(j + 1) * 512],
                             start=True, stop=True)
            nc.vector.tensor_copy(colacc[:, j * 512:(j + 1) * 512], pc[:])
        # ReduceScatter the seq-row column contributions: rank r's output
        # shard is rows [r*768, (r+1)*768) = exactly our local seq rows.
        rs_in = dram_pool.tile([HALL], F32, tag="rs_in")
        rs_out = dram_pool.tile([HROWS], F32, tag="rs_out")
        nc.sync.dma_start(rs_in[:], colacc[:])
        nc.gpsimd.collective_compute(
            "ReduceScatter", ALU.add,
            replica_groups=[list(range(N_CORES))],
            ins=[rs_in.opt()], outs=[rs_out.opt()])
        for j in range(HIB):
            nc.sync.dma_start(colden[:, j:j + 1],
                              rs_out[j * 128:(j + 1) * 128])
        # phase B rest: seq rows x seq cols (row sums only)
        for ib in range(HIB + 1, IB):
            b_block(ib)

    # ---------- final reduction ----------
    with tc.tile_pool(name="ps_fin", bufs=1, space="PSUM") as ps_fin:
        denom = small_pool.tile([128, IB], F32, tag="denom")
        nc.vector.tensor_reduce(
            denom[:], stats.rearrange("p (i x) -> p i x", x=2 * CC),
            axis=mybir.AxisListType.X, op=ALU.add)
        # seq rows: add the ReduceScattered spa-column contributions
        nc.vector.tensor_add(denom[:, HIB:IB], denom[:, HIB:IB], colden[:])
        logd = small_pool.tile([128, IB], F32, tag="logd")
        nc.scalar.activation(logd[:], denom[:], AF.Ln, bias=neg_e2[:])
        logsum = small_pool.tile([128, 1], F32, tag="logsum")
        nc.vector.tensor_reduce(logsum[:], logd[:],
                                axis=mybir.AxisListType.X, op=ALU.add)
        fin = ps_fin.tile([1, 1], F32, tag="fin")
        nc.tensor.matmul(fin[:], lhsT=ones_col[:], rhs=logsum[:],
                         start=True, stop=True)
        res = small_pool.tile([1, 1], F32, tag="res")
        # res = (pos_sum * POS_COEF + sum(log denom)) / R
        nc.vector.scalar_tensor_tensor(res[:], pos_sum[:], POS_COEF,
                                       fin[:], op0=ALU.mult, op1=ALU.add)
        nc.vector.tensor_scalar_mul(res[:], res[:], INV_COUNT)
        nc.sync.dma_start(out_ap[:], res[:])


_NC_CACHE = None


def build_nc():
    global _NC_CACHE
    if _NC_CACHE is not None:
        return _NC_CACHE
    nc = bacc.Bacc("TRN2", target_bir_lowering=False, debug=False,
                   num_devices=N_CORES)
    f_spa = nc.dram_tensor("f_spa", [BL, 4, D], F32, kind="ExternalInput").ap()
    f_seq = nc.dram_tensor("f_seq", [BL, 4, D], F32, kind="ExternalInput").ap()
    w_ap = nc.dram_tensor("W", [2 * D, D], F32, kind="ExternalInput").ap()
    b_ap = nc.dram_tensor("b", [D], F32, kind="ExternalInput").ap()
    out_ap = nc.dram_tensor("out", [1, 1], F32, kind="ExternalOutput").ap()
    with tile.TileContext(nc) as tc, ExitStack() as ctx:
        _body(ctx, nc, tc, (f_spa, f_seq), w_ap, b_ap, out_ap)
    nc.compile()
    _NC_CACHE = nc
    return nc


def run(inputs, **kw):
    nc = build_nc()
    f_seq = np.ascontiguousarray(np.asarray(inputs["f_seq"], dtype=np.float32))
    f_spa = np.ascontiguousarray(np.asarray(inputs["f_spa"], dtype=np.float32))
    W = np.ascontiguousarray(np.asarray(inputs["W"], dtype=np.float32))
    b = np.ascontiguousarray(np.asarray(inputs["b"], dtype=np.float32))
    in_maps = []
    for c in range(N_CORES):
        sl = slice(c * BL, (c + 1) * BL)
        in_maps.append({"f_seq": np.ascontiguousarray(f_seq[sl]),
                        "f_spa": np.ascontiguousarray(f_spa[sl]),
                        "W": W, "b": b})
    try:
        res = bass_utils.run_bass_kernel_spmd(
            nc, in_maps, core_ids=list(range(N_CORES)), **kw)
    except Exception:
        # the axon terminal occasionally reports a transient
        # "device unrecoverable" on first attach; one retry clears it
        import time
        time.sleep(15)
        res = bass_utils.run_bass_kernel_spmd(
            nc, in_maps, core_ids=list(range(N_CORES)), **kw)
    total = np.float64(0.0)
    for c in range(N_CORES):
        total += np.float64(res.results[c]["out"][0, 0])
    return np.float32(total), res


def kernel(**inputs) -> np.ndarray:
    loss, _ = run(inputs)
    return np.asarray(loss, dtype=np.float32)


if __name__ == "__main__":
    rng = np.random.default_rng(0)
    inputs = {
        "f_seq": rng.standard_normal((B, 4, D), dtype=np.float32),
        "f_spa": rng.standard_normal((B, 4, D), dtype=np.float32),
        "W": (rng.standard_normal((2 * D, D), dtype=np.float32) * 0.02),
        "b": np.zeros((D,), dtype=np.float32),
    }
    print(kernel(**inputs))

